# revision 1
# baseline (speedup 1.0000x reference)
"""Trainium2 Bass kernel for nn_CodaAttention (GQA attention with depth-KV
prefix, QK-norm, RoPE, XSA value-projection subtraction).

Sharding: tensor-parallel over heads across 8 cores. Core c owns q-heads
{2c, 2c+1} and kv-head c//2 (each kv head's k/v projection is duplicated on
2 cores). Inputs are pre-transposed on the host into the device-friendly
layouts (x^T [dim, tok], w^T [dim, out]); x^T is cast to bf16 (sharded along
dim) and AllGathered. Attention uses transposed logits L^T[k, q] so the
softmax'd probabilities come out directly in the lhsT layout needed by the
PV matmul (no P transposes), and QK-norm bounds |logits| <= sqrt(128) so no
max-subtraction is needed. V in [tok, hd] layout is recovered from the
computed v^T via a DRAM roundtrip + DMA-transpose on a seq-aligned key grid
(the depth-KV prefix is a separate M=64 key tile). After attention + XSA
each core AllGathers y^T (bf16) and computes its 256-row output slice of wo.
"""
import os
import sys

sys.path.insert(0, "/opt/trn_rl_repo")

import numpy as np

import concourse.bass as bass
import concourse.mybir as mybir
import concourse.tile as tile
from concourse import bacc

DT = mybir.dt
F32, BF16 = DT.float32, DT.bfloat16
AF = mybir.ActivationFunctionType
ALU = mybir.AluOpType

B, T, DIM = 2, 2048, 2048
H, KVH, HD = 16, 4, 128
TD = 64
NCORES = 8
HPC = H // NCORES            # q heads per core = 2
TOK = B * T                  # 4096 flattened tokens
DPC = DIM // NCORES          # 256 dim-rows per core (x^T cast shard)
NKD = DIM // 128             # 16 contraction tiles
SCALE = 1.0 / np.sqrt(HD)

_DBG = bool(int(os.environ.get("KERNEL_DEBUG_DUMPS", "0")))
_PH = int(os.environ.get("KERNEL_PHASES", "3"))
_ABL = set(os.environ.get("KERNEL_ABLATE", "").split(","))
_REP = int(os.environ.get("KERNEL_REPEAT", "1"))
_SINGLE = bool(int(os.environ.get("KERNEL_SINGLE", "0")))


def _build():
    nc = bacc.Bacc("TRN2", target_bir_lowering=False, debug=False,
                   num_devices=1 if _SINGLE else NCORES)

    # ---------------- I/O (all host-side layouts are pre-transposed) -------
    def inp(name, shape):
        return nc.dram_tensor(name, list(shape), F32, kind="ExternalInput").ap()

    xT_sh = inp("xT_sh", (DPC, TOK))           # this core's dim-slice of x^T
    xT_full = inp("xT_full", (DIM, TOK)) if _SINGLE else None
    wqT_c = inp("wqT_c", (DIM, HPC * HD))      # wq slice, transposed
    wkT_c = inp("wkT_c", (DIM, HD))
    wvT_c = inp("wvT_c", (DIM, HD))
    woT_c = inp("woT_c", (DIM, HPC * HD))      # wo out-row slice, transposed
    vbT_c = inp("vbT_c", (HD, TOK))            # transposed value_bias slice
    dkT_c = inp("dkT_c", (B, HD, TD))          # transposed depth_k slice
    dv_c = inp("dv_c", (B, TD, HD))
    cosT = inp("cosT", (HD, T))                # pair-duplicated cos, [128, 2048]
    sinT = inp("sinT", (HD, T))                # pair-duplicated sign-folded sin
    qs_c = inp("qs_c", (128, HPC))             # q_scale per local head, bcast
    ks_c = inp("ks_c", (128, 1))               # k_scale, bcast

    outT = nc.dram_tensor("outT", [HPC * HD, TOK], F32,
                          kind="ExternalOutput").ap()

    # ---------------- DRAM scratch ----------------
    xT_bf_sh = [nc.dram_tensor(f"xT_bf_sh{i}", [DPC, TOK // 4], BF16).ap()
                for i in range(4)]
    xT_bf = [nc.dram_tensor(f"xT_bf{i}", [DIM, TOK // 4], BF16,
                            addr_space="Shared").ap() for i in range(4)]
    wqT_bf = nc.dram_tensor("wqT_bf", [DIM, HPC * HD], BF16).ap()
    wkT_bf = nc.dram_tensor("wkT_bf", [DIM, HD], BF16).ap()
    wvT_bf = nc.dram_tensor("wvT_bf", [DIM, HD], BF16).ap()
    woT_bf = nc.dram_tensor("woT_bf", [DIM, HPC * HD], BF16).ap()
    vt_dram = nc.dram_tensor("vt_dram", [B, HD, T], BF16).ap()
    y_mine = [nc.dram_tensor(f"y_mine{i}", [HPC * HD, TOK // 4], BF16).ap()
              for i in range(4)]
    y_all = [nc.dram_tensor(f"y_all{i}", [H * HD, TOK // 4], BF16,
                            addr_space="Shared").ap() for i in range(4)]

    dbg = {}
    if _DBG:
        dbg["qT"] = nc.dram_tensor("dbg_qT", [HPC, B, HD, T], F32,
                                   kind="ExternalOutput").ap()
        dbg["kT"] = nc.dram_tensor("dbg_kT", [B, HD, TD + T], F32,
                                   kind="ExternalOutput").ap()
        dbg["v"] = nc.dram_tensor("dbg_v", [B, 128, 16 * 128], F32,
                                  kind="ExternalOutput").ap()
        dbg["y"] = nc.dram_tensor("dbg_y", [HPC * HD, TOK], F32,
                                  kind="ExternalOutput").ap()

    with tile.TileContext(nc) as tc:
        for _rep in range(_REP):
            _emit_once(nc, tc, locals())
    nc.compile()
    return nc


def _emit_once(nc, tc, v):
    xT_sh, wqT_c, wkT_c, wvT_c, woT_c = (v["xT_sh"], v["wqT_c"], v["wkT_c"],
                                         v["wvT_c"], v["woT_c"])
    vbT_c, dkT_c, dv_c, cosT, sinT = (v["vbT_c"], v["dkT_c"], v["dv_c"],
                                      v["cosT"], v["sinT"])
    qs_c, ks_c, outT = v["qs_c"], v["ks_c"], v["outT"]
    xT_bf_sh, xT_bf = v["xT_bf_sh"], v["xT_bf"]
    wqT_bf, wkT_bf, wvT_bf, woT_bf = (v["wqT_bf"], v["wkT_bf"], v["wvT_bf"],
                                      v["woT_bf"])
    vt_dram, y_mine, y_all, dbg = v["vt_dram"], v["y_mine"], v["y_all"], v["dbg"]

    # =========== P0: casts, chunked AllGather(x^T), weights, constants =====
    for i in range(4):  # token-chunked cast + AG pipeline
        nc.gpsimd.dma_start(out=xT_bf_sh[i][:, :],
                            in_=xT_sh[:, 1024 * i:1024 * (i + 1)])
        if _SINGLE:
            # model substitute: same per-core DMA work as the real cast;
            # the gather itself runs on SDMA/TOPSP (not engine time)
            nc.gpsimd.dma_start(
                out=xT_bf[i][0:DPC, :],
                in_=v["xT_full"][0:DPC, 1024 * i:1024 * (i + 1)])
        else:
            nc.gpsimd.collective_compute(
                "AllGather", ALU.bypass,
                replica_groups=[list(range(NCORES))],
                ins=[xT_bf_sh[i][:, :]], outs=[xT_bf[i][:, :]])
    nc.gpsimd.dma_start(out=wqT_bf[:, :], in_=wqT_c[:, :])
    nc.gpsimd.dma_start(out=wkT_bf[:, :], in_=wkT_c[:, :])
    nc.gpsimd.dma_start(out=wvT_bf[:, :], in_=wvT_c[:, :])
    nc.gpsimd.dma_start(out=woT_bf[:, :], in_=woT_c[:, :])

    const = tc.alloc_tile_pool(name="const", bufs=1)
    wpool = tc.alloc_tile_pool(name="wpool", bufs=1)
    big = tc.alloc_tile_pool(name="big", bufs=1)

    cos_sb = const.tile([HD, T], BF16, tag="cos")
    sin_sb = const.tile([HD, T], BF16, tag="sin")
    nc.gpsimd.dma_start(out=cos_sb[:, :], in_=cosT[:, :])
    nc.gpsimd.dma_start(out=sin_sb[:, :], in_=sinT[:, :])
    qs_sb = const.tile([128, HPC], F32, tag="qs")
    ks_sb = const.tile([128, 1], F32, tag="ks")
    nc.sync.dma_start(out=qs_sb[:, :], in_=qs_c[:, :])
    nc.sync.dma_start(out=ks_sb[:, :], in_=ks_c[:, :])
    ones_bf = const.tile([128, 128], BF16, tag="ones")
    nc.gpsimd.memset(ones_bf[:, :], 1.0)
    eps_sb = const.tile([128, 1], F32, tag="eps")
    nc.gpsimd.memset(eps_sb[:, :], 1e-12)
    # 0/1 causal masks, keep where c >= p + d.
    # masks[0] (d=0): depth tile for query group 0 (j = p).
    # masks[1..5] (d=128jj-64): seq tiles straddling the causal boundary;
    # the seq grid is shifted +64 vs queries so FIVE tiles need masking.
    masks = []
    for mi, d in enumerate((0, -64, 64, 192, 320, 448)):
        m = const.tile([128, 512], BF16, tag=f"mask{mi}", name=f"mask{mi}")
        nc.gpsimd.memset(m[:, :], 1.0)
        nc.gpsimd.affine_select(out=m[:, :], in_=m[:, :],
                                compare_op=ALU.is_ge, fill=0.0,
                                base=-d, channel_multiplier=-1,
                                pattern=[[1, 512]])
        masks.append(m)
    # combined masks for the augmented tile: rows 0:64 = d448 seq pattern,
    # rows 64:128 = depth (triangle for group 0, all-keep otherwise)
    maskA = const.tile([128, 512], BF16, tag="maskA", name="maskA")
    maskB = const.tile([128, 512], BF16, tag="maskB", name="maskB")
    nc.vector.tensor_copy(maskA[0:TD, :], masks[5][0:TD, :])
    nc.vector.tensor_copy(maskA[TD:128, :], masks[1][TD:128, :])
    nc.vector.tensor_copy(maskB[0:TD, :], masks[5][0:TD, :])
    nc.gpsimd.memset(maskB[TD:128, :], 1.0)

    # weight lhsT tiles [128 dim, 128 out] -- plain loads, layout is already
    # transposed in DRAM
    def wtiles(src_bf, nrow_tiles, tag):
        ts = []
        for m in range(nrow_tiles):
            row = []
            for kk in range(NKD):
                t = wpool.tile([128, 128], BF16, tag=f"{tag}{m}_{kk}",
                               name=f"{tag}{m}_{kk}")
                nc.sync.dma_start(
                    out=t[:, :],
                    in_=src_bf[128 * kk:128 * (kk + 1), 128 * m:128 * (m + 1)])
                row.append(t)
            ts.append(row)
        return ts

    wqT = wtiles(wqT_bf, HPC, "wq")        # [2][16] tiles
    wkT = wtiles(wkT_bf, 1, "wk")[0]       # [16]
    wvT = wtiles(wvT_bf, 1, "wv")[0]
    woT = wtiles(woT_bf, HPC, "wo")        # [2][16]

    # big persistent activations
    QT = [[big.tile([HD, T], BF16, tag=f"QT{h}_{b}", name=f"QT{h}_{b}")
           for b in range(B)] for h in range(HPC)]
    KT = [big.tile([HD, TD + T], BF16, tag=f"KT{b}", name=f"KT{b}")
          for b in range(B)]
    VC = [big.tile([128, 16 * 128], BF16, tag=f"VC{b}", name=f"VC{b}")
          for b in range(B)]
    VCd = [big.tile([TD, HD], BF16, tag=f"VCd{b}", name=f"VCd{b}")
           for b in range(B)]
    VTs = [big.tile([HD, T], BF16, tag=f"VTs{b}", name=f"VTs{b}")
           for b in range(B)]

    for b in range(B):
        # depth parts (cast f32->bf16 in the DMA)
        nc.gpsimd.dma_start(out=KT[b][:, 0:TD], in_=dkT_c[b, :, :])
        nc.gpsimd.dma_start(out=VCd[b][:, :], in_=dv_c[b, :, :])
    # augmented last-diagonal tiles: [live 64 seq keys | 64 depth keys]
    KTa = [[big.tile([HD, 128], BF16, tag=f"KTa{b}_{g}", name=f"KTa{b}_{g}")
            for g in range(4)] for b in range(B)]
    VCa = [[big.tile([128, HD], BF16, tag=f"VCa{b}_{g}", name=f"VCa{b}_{g}")
            for g in range(4)] for b in range(B)]
    for b in range(B):
        for g in range(4):
            nc.gpsimd.dma_start(out=VCa[b][g][TD:128, :], in_=dv_c[b, :, :])

    # =========== P1: projections + rope + qk-norm + v^T ====================
    rp = tc.alloc_tile_pool(name="rope", bufs=3)
    xp = tc.alloc_tile_pool(name="xT", bufs=1)
    vbp = tc.alloc_tile_pool(name="vb", bufs=3)
    pp = tc.alloc_tile_pool(name="pproj", bufs=4, space="PSUM")
    pps = tc.alloc_tile_pool(name="pss", bufs=2, space="PSUM")

    def rope_norm(ps, b, n, scale_ap, out_ap):
        """psum [128,512] raw head-dim-major proj -> rope -> l2norm*scale ->
        bf16 out_ap."""
        cs = cos_sb[:, 512 * n:512 * (n + 1)]
        sn = sin_sb[:, 512 * n:512 * (n + 1)]
        qb = rp.tile([128, 512], BF16, tag="qb", name="qb")
        nc.vector.tensor_copy(qb[:, :], ps[:, :])
        swp = rp.tile([128, 512], BF16, tag="swp", name="swp")
        mask32 = []
        for j in range(16):
            mask32 += [2 * j + 1, 2 * j]
        nc.vector.stream_shuffle(swp[:, :], qb[:, :], mask32)
        m1 = rp.tile([128, 512], BF16, tag="m1", name="m1")
        nc.vector.tensor_mul(m1[:, :], qb[:, :], cs)
        m2 = rp.tile([128, 512], BF16, tag="m2", name="m2")
        nc.vector.tensor_mul(m2[:, :], swp[:, :], sn)
        qr = rp.tile([128, 512], BF16, tag="qr", name="qr")
        nc.vector.tensor_add(qr[:, :], m1[:, :], m2[:, :])
        q2 = rp.tile([128, 512], BF16, tag="q2", name="q2")
        nc.vector.tensor_mul(q2[:, :], qr[:, :], qr[:, :])
        ss = pps.tile([128, 512], F32, tag="ss", name="ss")
        nc.tensor.matmul(ss[:, :], ones_bf[:, :], q2[:, :], start=True,
                         stop=True)
        nrm = rp.tile([128, 512], F32, tag="nrm", name="nrm")
        nc.scalar.activation(nrm[:, :], ss[:, :], AF.Sqrt, bias=eps_sb[:, :])
        ri = rp.tile([128, 512], F32, tag="ri", name="ri")
        nc.vector.reciprocal(ri[:, :], nrm[:, :])
        nc.vector.scalar_tensor_tensor(out_ap, qr[:, :], scale_ap, ri[:, :],
                                       op0=ALU.mult, op1=ALU.mult)

    for b in range(B):
        for n in range(T // 512):  # 4 chunks of 512 tokens
            r0 = b * T + 512 * n
            if n % 2 == 0:  # load 1024-token x^T stripes (2 chunks worth)
                ci = r0 // 1024
                xt2 = [xp.tile([128, 1024], BF16, tag=f"xt{kk}",
                               name=f"xt{kk}") for kk in range(NKD)]
                for kk in range(NKD):
                    nc.sync.dma_start(
                        out=xt2[kk][:, :],
                        in_=xT_bf[ci][128 * kk:128 * (kk + 1), :])
            off = (n % 2) * 512
            xt = [xt2[kk][:, off:off + 512] for kk in range(NKD)]
            # q heads
            for h in range(HPC):
                ps = pp.tile([128, 512], F32, tag="pq", name="psq")
                for kk in range(NKD):
                    nc.tensor.matmul(ps[:, :], wqT[h][kk][:, :], xt[kk][:, :],
                                     start=(kk == 0), stop=(kk == NKD - 1))
                rope_norm(ps, b, n, qs_sb[:, h:h + 1],
                          QT[h][b][:, 512 * n:512 * (n + 1)])
            # k
            ps = pp.tile([128, 512], F32, tag="pq", name="psk")
            for kk in range(NKD):
                nc.tensor.matmul(ps[:, :], wkT[kk][:, :], xt[kk][:, :],
                                 start=(kk == 0), stop=(kk == NKD - 1))
            rope_norm(ps, b, n, ks_sb[:, 0:1],
                      KT[b][:, TD + 512 * n:TD + 512 * (n + 1)])
            # v^T (head-dim-major); V natural is recovered via DMA-transpose
            pvt = pp.tile([128, 512], F32, tag="pq", name="pvt")
            for kk in range(NKD):
                nc.tensor.matmul(pvt[:, :], wvT[kk][:, :], xt[kk][:, :],
                                 start=(kk == 0), stop=(kk == NKD - 1))
            vbt_sb = vbp.tile([128, 512], F32, tag="vbts", name="vbt_sb")
            nc.scalar.dma_start(out=vbt_sb[:, :], in_=vbT_c[:, r0:r0 + 512])
            nc.vector.tensor_add(VTs[b][:, 512 * n:512 * (n + 1)],
                                 pvt[:, :], vbt_sb[:, :])
            nc.scalar.dma_start(out=vt_dram[b, :, 512 * n:512 * (n + 1)],
                                in_=VTs[b][:, 512 * n:512 * (n + 1)])
        # batched xbar window: all V transposes for this b at once
        for tt in range(16):
            nc.sync.dma_start(out=VC[b][:, 128 * tt:128 * (tt + 1)],
                              in_=vt_dram[b, :, 128 * tt:128 * (tt + 1)],
                              transpose=True)
        for g in range(4):
            s0 = TD + 512 * g + 384
            nc.vector.tensor_copy(KTa[b][g][:, 0:TD], KT[b][:, s0:s0 + TD])
            nc.vector.tensor_copy(KTa[b][g][:, TD:128], KT[b][:, 0:TD])
            nc.vector.tensor_copy(VCa[b][g][0:TD, :],
                                  VC[b][0:TD, 128 * (4 * g + 3):128 * (4 * g + 4)])

    for p in (pps, pp, vbp, xp):
        p.release()

    if dbg:
        def dump(dst_ap, src_ap, width):
            for j0 in range(0, width, 512):
                w = min(512, width - j0)
                sb = rp.tile([128, 512], F32, tag="dbgc", name="dbgc")
                nc.vector.tensor_copy(sb[:, 0:w], src_ap[:, j0:j0 + w])
                nc.scalar.dma_start(out=dst_ap[:, j0:j0 + w], in_=sb[:, 0:w])
        for h in range(HPC):
            for b in range(B):
                dump(dbg["qT"][h, b, :, :], QT[h][b][:, :], T)
        for b in range(B):
            dump(dbg["kT"][b, :, :], KT[b][:, :], TD + T)
            dump(dbg["v"][b, :, :], VC[b][:, :], 16 * 128)

    if _PH < 2:
        for p in (rp, big, wpool, const):
            p.release()
        return

    # =========== P2+P3: attention + XSA per (b, g), heads inner =============
    ap_sb = tc.alloc_tile_pool(name="attn_sb", bufs=2)
    vt_sb = tc.alloc_tile_pool(name="vt_sb", bufs=1)
    ppl = tc.alloc_tile_pool(name="pL", bufs=2, space="PSUM")
    ppy = tc.alloc_tile_pool(name="pY", bufs=2, space="PSUM")
    ppz = tc.alloc_tile_pool(name="pZ", bufs=2, space="PSUM")
    ppvn = tc.alloc_tile_pool(name="pVn", bufs=1, space="PSUM")
    ppd = tc.alloc_tile_pool(name="pD", bufs=1, space="PSUM")

    for b in range(B):
        for g in range(4):
            nk = 4 * (g + 1)  # seq k-tiles of 128
            # --- v_seq^T for this query group: direct slice of VT_seq ---
            skip_xsa = "vt" in _ABL
            vTg = VTs[b][:, 512 * g:512 * (g + 1)]
            rv = vt_sb.tile([128, 512], F32, tag="rv", name="rv")
            if not skip_xsa:
                v2 = vt_sb.tile([128, 512], BF16, tag="v2", name="v2")
                nc.vector.tensor_mul(v2[:, :], vTg, vTg)
                vns = ppvn.tile([128, 512], F32, tag="vns", name="vns")
                nc.tensor.matmul(vns[:, :], ones_bf[:, :], v2[:, :],
                                 start=True, stop=True)
                nc.vector.reciprocal(rv[:, :], vns[:, :])

            for h in range(HPC):
                q_sl = QT[h][b][:, 512 * g:512 * (g + 1)]
                y_ps = ppy.tile([128, 512], F32, tag="y", name="y_ps")
                z_ps = ppz.tile([128, 512], F32, tag="z", name="z_ps")
                for kt in range(nk):
                    last = kt == nk - 1
                    kT_t = (KTa[b][g][:, :] if last else
                            KT[b][:, TD + 128 * kt:TD + 128 * (kt + 1)])
                    v_t = (VCa[b][g][:, :] if last else
                           VC[b][:, 128 * kt:128 * kt + HD])
                    L = ppl.tile([128, 512], F32, tag="L", name="L")
                    nc.tensor.matmul(L[:, :], kT_t, q_sl, start=True,
                                     stop=True)
                    P = ap_sb.tile([128, 512], BF16, tag="P", bufs=4, name="P")
                    nc.scalar.activation(P[:, :], L[:, :], AF.Exp, scale=SCALE)
                    di = kt - 4 * g
                    if "mask" not in _ABL:
                        if last:
                            nc.vector.tensor_mul(
                                P[:, :], P[:, :],
                                (maskA if g == 0 else maskB)[:, :])
                        elif di >= -1:
                            nc.vector.tensor_mul(P[:, :], P[:, :],
                                                 masks[di + 2][:, :])
                    nc.tensor.matmul(z_ps[:, :], ones_bf[:, :], P[:, :],
                                     start=(kt == 0), stop=last)
                    nc.tensor.matmul(y_ps[:, :], v_t, P[:, :],
                                     start=(kt == 0), stop=last)
                # softmax denom + XSA
                rz = ap_sb.tile([128, 512], F32, tag="rz", name="rz")
                nc.vector.reciprocal(rz[:, :], z_ps[:, :])
                yf = ap_sb.tile([128, 512], BF16, tag="yf", name="yf")
                if skip_xsa:
                    nc.vector.tensor_mul(yf[:, :], y_ps[:, :], rz[:, :])
                else:
                    yv = ap_sb.tile([128, 512], BF16, tag="yv", name="yv")
                    nc.vector.tensor_mul(yv[:, :], y_ps[:, :], vTg)
                    dot = ppd.tile([128, 512], F32, tag="dot", name="dot")
                    nc.tensor.matmul(dot[:, :], ones_bf[:, :], yv[:, :],
                                     start=True, stop=True)
                    coef = ap_sb.tile([128, 512], F32, tag="coef", name="coef")
                    nc.vector.tensor_mul(coef[:, :], dot[:, :], rv[:, :])
                    t1 = ap_sb.tile([128, 512], F32, tag="t1", name="t1")
                    nc.vector.tensor_mul(t1[:, :], coef[:, :], vTg)
                    y1 = ap_sb.tile([128, 512], F32, tag="y1", name="y1")
                    nc.vector.tensor_sub(y1[:, :], y_ps[:, :], t1[:, :])
                    nc.vector.tensor_mul(yf[:, :], y1[:, :], rz[:, :])
                col = b * T + 512 * g
                nc.scalar.dma_start(
                    out=y_mine[col // 1024][128 * h:128 * (h + 1),
                               col % 1024:col % 1024 + 512],
                    in_=yf[:, :])

    for p in (ppd, ppvn, ppz, ppy, ppl):
        p.release()

    if dbg:
        for h in range(HPC):
            for j in range(TOK // 512):
                t = ap_sb.tile([HD, 512], BF16, tag="dbgyb", name="dbgyb")
                nc.scalar.dma_start(
                    out=t[:, :],
                    in_=y_mine[j // 2][128 * h:128 * (h + 1),
                                       512 * (j % 2):512 * (j % 2 + 1)])
                sbt = ap_sb.tile([HD, 512], F32, tag="dbgy", name="dbgy")
                nc.vector.tensor_copy(sbt[:, :], t[:, :])
                nc.scalar.dma_start(
                    out=dbg["y"][128 * h:128 * (h + 1), 512 * j:512 * (j + 1)],
                    in_=sbt[:, :])

    if _PH < 3:
        for p in (vt_sb, ap_sb, rp, big, wpool, const):
            p.release()
        return

    # =========== P4: AllGather(y) + wo ======================================
    for i in range(4):
        if _SINGLE:
            nc.scalar.dma_start(out=y_all[i][0:256, :], in_=y_mine[i][:, :])
        elif True:
            nc.gpsimd.collective_compute(
                "AllGather", ALU.bypass, replica_groups=[list(range(NCORES))],
                ins=[y_mine[i][:, :]], outs=[y_all[i][:, :]])

    wop = tc.alloc_tile_pool(name="wo_rhs", bufs=4)
    wos = tc.alloc_tile_pool(name="wo_sb", bufs=3)
    ppo = tc.alloc_tile_pool(name="pO", bufs=1, space="PSUM")
    NCT = H * HD // 128  # 16 contraction tiles
    for qg in range(4):  # 1024-token chunks, matching y_all chunking
        po = [[ppo.tile([128, 512], F32, tag=f"po{m}_{s}", name=f"po{m}_{s}")
               for s in range(2)] for m in range(HPC)]
        for cc in range(NCT):
            t = wop.tile([128, 1024], BF16, tag=f"yr{cc % 4}", name=f"yr{cc}")
            nc.scalar.dma_start(out=t[:, :],
                                in_=y_all[qg][128 * cc:128 * (cc + 1), :])
            for m in range(HPC):
                for s in range(2):
                    nc.tensor.matmul(po[m][s][:, :], woT[m][cc][:, :],
                                     t[:, 512 * s:512 * (s + 1)],
                                     start=(cc == 0), stop=(cc == NCT - 1))
        for m in range(HPC):
            for s in range(2):
                ob = wos.tile([128, 512], F32, tag="ob", name="ob")
                nc.vector.tensor_copy(ob[:, :], po[m][s][:, :])
                nc.scalar.dma_start(
                    out=outT[128 * m:128 * (m + 1),
                             1024 * qg + 512 * s:1024 * qg + 512 * (s + 1)],
                    in_=ob[:, :])

    for p in (ppo, wos, wop, vt_sb, ap_sb, rp, big, wpool, const):
        p.release()


_NC_CACHE = None


def _get_nc():
    global _NC_CACHE
    if _NC_CACHE is None:
        _NC_CACHE = _build()
    return _NC_CACHE


def _shard_inputs(inputs):
    x = np.asarray(inputs["x"], np.float32)
    fc = np.asarray(inputs["freqs_cos"], np.float32)
    fs = np.asarray(inputs["freqs_sin"], np.float32)
    vb = np.asarray(inputs["value_bias"], np.float32)
    dk = np.asarray(inputs["depth_k"], np.float32)
    dv = np.asarray(inputs["depth_v"], np.float32)
    wq = np.asarray(inputs["wq"], np.float32)
    wk = np.asarray(inputs["wk"], np.float32)
    wv = np.asarray(inputs["wv"], np.float32)
    wo = np.asarray(inputs["wo"], np.float32)
    qs = np.asarray(inputs["q_scale"], np.float32).reshape(H)
    ks = np.asarray(inputs["k_scale"], np.float32).reshape(KVH)

    xT = np.ascontiguousarray(x.reshape(TOK, DIM).T)     # [DIM, TOK]
    cosT = np.ascontiguousarray(np.repeat(fc.T, 2, axis=0))
    sinT = np.repeat(fs.T, 2, axis=0).copy()
    sinT[0::2] *= -1.0
    sinT = np.ascontiguousarray(sinT)
    vbf = vb.reshape(TOK, KVH * HD)

    maps = []
    for c in range(NCORES):
        kvh = c // 2
        m = {
            "xT_sh": np.ascontiguousarray(xT[DPC * c:DPC * (c + 1)]),
            "wqT_c": np.ascontiguousarray(wq[256 * c:256 * (c + 1)].T),
            "wkT_c": np.ascontiguousarray(wk[HD * kvh:HD * (kvh + 1)].T),
            "wvT_c": np.ascontiguousarray(wv[HD * kvh:HD * (kvh + 1)].T),
            "woT_c": np.ascontiguousarray(wo[256 * c:256 * (c + 1)].T),
            "vbT_c": np.ascontiguousarray(
                vbf[:, HD * kvh:HD * (kvh + 1)].T),
            "dkT_c": np.ascontiguousarray(dk[:, kvh].transpose(0, 2, 1)),
            "dv_c": np.ascontiguousarray(dv[:, kvh]),
            "cosT": cosT,
            "sinT": sinT,
            "qs_c": np.ascontiguousarray(
                np.broadcast_to(qs[2 * c:2 * c + 2][None, :], (128, 2))),
            "ks_c": np.full((128, 1), ks[kvh], np.float32),
        }
        maps.append(m)
    return maps


def _gather_output(results):
    outT = np.concatenate([results[c]["outT"] for c in range(NCORES)], axis=0)
    return np.ascontiguousarray(outT.T).reshape(B, T, DIM).astype(np.float32)


def kernel(**inputs):
    from concourse import bass_utils
    nc = _get_nc()
    from concourse.bass_interp import get_hw_module
    maps = _shard_inputs(inputs)
    old = nc.m
    nc.m = get_hw_module(nc.m)
    try:
        res = bass_utils.run_bass_kernel_spmd(nc, maps, list(range(NCORES)))
    finally:
        nc.m = old
    return _gather_output(res.results)

